# revision 23
# baseline (speedup 1.0000x reference)
"""Distributed Node-GCN forward on 8 Trainium2 NeuronCores (Bass/Tile).

Strategy (1-D node partition, v3):
  - Host reassigns the 50000 nodes to 8 cores x 6912 table slots (72 blocks
    of 96 rows).  Node-id halves map to slot halves (slots <3456 vs >=3456)
    so the per-round table replication splits into TWO AllGathers of
    [8*3456, 128] each, double-buffered by round parity and issued
    mid-previous-round so they hide behind the gather stream.
  - Each spmm = bulk dma_gather of r[col] rows (256B fp16 table rows) +
    one PE matmul per 128-edge chunk with a host-precomputed selector
    matrix SelT[128,32] performing val-scaling + segment-sum into a
    32-row window of a 96-row PSUM block.
  - Gather descriptor generation is spread over 4 SWDGE queues (4 Q7 core
    pairs run concurrently).  Col-half-1 gathers are issued KDELAY calls
    behind col-half-0 so the round's second AllGather can complete without
    stalling the gpsimd sequencer.
  - Edge structure (indices + selectors) is shared by all 4 spmms and kept
    resident in SBUF.
"""
import numpy as np

N = 50000
FEAT = 256
H = 64
CLASS = 64
CORES = 8
BLK = 96            # PSUM block rows
NBLK = 72           # blocks per core
SHARD = BLK * NBLK  # 6912 table slots per core
WIN = 32            # selector window rows
NWPB = 3            # windows per block
NWIN = NBLK * NWPB  # 216 windows per core
HWIN = NWIN // 2    # 108 windows per slot-half
HS = SHARD // 2     # 3456 slots per half
REP = 2             # chunks per (window, col-half)
NCH = NWIN * REP    # 432 chunks per (core, col-half)
HALFROWS = CORES * HS    # 27648 rows per table half
SB = 1                   # blocks per gather call
NCALL = NBLK // SB       # 72 calls per (spmm, half)
CALL_IDX = SB * NWPB * REP * 128   # 768 idxs per call
ELEM = 128               # table row elements (fp16) = 256B
KC = 512                 # padded classify contraction (448 -> 512)
KDELAY = 20              # col-half-1 gather issue delay (calls)
FLUSH_BLKS = 6           # blocks per batched agin/strip flush

_CACHE = {}


# --------------------------------------------------------------------------
# Host preprocessing
# --------------------------------------------------------------------------
def _assign_rows(edge_rows, edge_cols):
    """Assign each node to (core, slot). Nodes < N/2 get slots < HS (half 0),
    the rest slots >= HS, on any core.  Within each half, greedily balance
    per-(core,window) edge loads (split by col half) under caps."""
    col_half = (edge_cols >= (N // 2)).astype(np.int64)
    a_cnt = np.bincount(edge_rows[col_half == 0], minlength=N)
    b_cnt = np.bincount(edge_rows[col_half == 1], minlength=N)

    core_of = np.empty(N, np.int64)
    slot_of = np.empty(N, np.int64)
    cap = REP * 128

    for half, lo in ((0, 0), (1, N // 2)):
        nodes = np.arange(lo, lo + N // 2)
        order = np.argsort(-(a_cnt[nodes] + b_cnt[nodes]), kind="stable")
        nodes = nodes[order]
        nbin = CORES * HWIN
        loadA = np.zeros(nbin, np.int64)
        loadB = np.zeros(nbin, np.int64)
        rows = np.zeros(nbin, np.int64)
        for nd in nodes:
            a, b = a_cnt[nd], b_cnt[nd]
            feas = (loadA + a <= cap) & (loadB + b <= cap) & (rows < WIN)
            score = np.where(feas, np.maximum(loadA + a, loadB + b), 1 << 40)
            j = int(np.argmin(score))
            if not feas[j]:
                feas2 = rows < WIN
                score2 = np.where(
                    feas2, np.maximum(loadA + a, loadB + b), 1 << 40)
                j = int(np.argmin(score2))
                if not feas2[j]:
                    raise RuntimeError("row-capacity infeasible")
            c = j // HWIN
            w = half * HWIN + (j % HWIN)
            core_of[nd] = c
            slot_of[nd] = w * WIN + rows[j]
            loadA[j] += a
            loadB[j] += b
            rows[j] += 1
    return core_of, slot_of


def _preprocess(edge_rows, edge_cols, edge_vals):
    core_of, slot_of = _assign_rows(edge_rows, edge_cols)
    e_core = core_of[edge_rows]
    e_half = (slot_of[edge_cols] >= HS).astype(np.int64)
    e_idx = core_of[edge_cols] * HS + (slot_of[edge_cols] % HS)
    e_slot = slot_of[edge_rows]
    e_win = e_slot // WIN
    e_lr = e_slot % WIN

    idx_arrs = np.zeros((CORES, 2, 128, NCH * 8), np.int16)
    sel_arrs = np.zeros((CORES, 2, 128, NCH * WIN), np.float16)

    for c in range(CORES):
        for h in range(2):
            m = (e_core == c) & (e_half == h)
            win = e_win[m]
            order = np.argsort(win, kind="stable")
            win = win[order]
            idxv = e_idx[m][order]
            lr = e_lr[m][order]
            val = edge_vals[m][order]
            counts = np.bincount(win, minlength=NWIN)
            if counts.max() > REP * 128:
                raise RuntimeError(f"window overflow: {counts.max()}")
            starts = np.concatenate([[0], np.cumsum(counts)[:-1]])
            pos = np.arange(len(win)) - starts[win]
            q = win * REP + pos // 128
            s_in_chunk = pos % 128
            gslot = q * 128 + s_in_chunk
            idx_flat = np.zeros(NCH * 128, np.int16)
            idx_flat[gslot] = idxv.astype(np.int16)
            wrapped = idx_flat.reshape(-1, 16).T
            idx_arrs[c, h] = np.tile(wrapped, (8, 1))
            sel = np.zeros((128, NCH * WIN), np.float16)
            sel[s_in_chunk, q * WIN + lr] = val.astype(np.float16)
            sel_arrs[c, h] = sel
    return core_of, slot_of, idx_arrs, sel_arrs


# --------------------------------------------------------------------------
# Workaround: this walrus build rejects >1 sync wait on a CTRL instruction;
# split the Tile kernel-tail drain's waits across multiple drains.
# --------------------------------------------------------------------------
def _patch_tile_drain():
    import concourse.tile as tile
    import concourse.mybir as mybir
    from concourse.vector_clock import ScopedClock

    if getattr(tile.TileContext, "_drain_split_patched", False):
        return

    def _patched(self, tick_clock, wait_clock):
        nc = self.nc
        drain_inst = nc.sync.drain()
        wait_clock.add_sem_waits(
            drain_inst.ins, ScopedClock({None: tick_clock.global_clock})
        )
        si = drain_inst.ins.sync_info
        if si is not None and len(si.on_wait) > 1:
            waits = list(si.on_wait)
            drain_inst.ins.sync_info = mybir.SyncInfo(
                on_wait=waits[:1], on_update=list(si.on_update))
            for i in range(1, len(waits)):
                extra = nc.sync.drain()
                extra.ins.sync_info = mybir.SyncInfo(
                    on_wait=[waits[i]], on_update=[])
        nc.all_engine_barrier()
        assert self.sems is not None
        popped = nc._tile_sem_poison_stack.pop()
        assert popped is self._sem_poison
        nc.clear_and_free_semaphores(list(self.sems.allocated().values()))
        nc.all_engine_barrier()

    tile.TileContext._drain_and_barrier = _patched
    tile.TileContext._drain_split_patched = True


# --------------------------------------------------------------------------
# Device kernel
# --------------------------------------------------------------------------
def _build_nc():
    import concourse.bacc as bacc
    import concourse.mybir as mybir
    import concourse.tile as tile
    from concourse import library_config

    _patch_tile_drain()

    fp16 = mybir.dt.float16
    fp32 = mybir.dt.float32
    i16 = mybir.dt.int16
    AX = mybir.AxisListType.X
    ALU = mybir.AluOpType
    ACT = mybir.ActivationFunctionType

    nc = bacc.Bacc("TRN2", target_bir_lowering=False, debug=False,
                   num_devices=CORES, num_swdge_queues=4)
    xT_d = nc.dram_tensor("xT", [FEAT, SHARD], fp16, kind="ExternalInput")
    we_d = nc.dram_tensor("we", [FEAT, H], fp16, kind="ExternalInput")
    wc_d = nc.dram_tensor("wc", [KC, CLASS], fp16, kind="ExternalInput")
    idx_d = [nc.dram_tensor(f"idx{h}", [128, NCH * 8], i16,
                            kind="ExternalInput") for h in range(2)]
    sel_d = [nc.dram_tensor(f"sel{h}", [128, NCH * WIN], fp16,
                            kind="ExternalInput") for h in range(2)]
    out_d = nc.dram_tensor("out", [SHARD, CLASS], fp32, kind="ExternalOutput")

    rg = [list(range(CORES))]

    with tile.TileContext(nc) as tc:
        nc.gpsimd.load_library(library_config.mlp)
        with (
            tc.tile_pool(name="dram", bufs=1, space="DRAM") as dram,
            tc.tile_pool(name="resident", bufs=1) as res,
            tc.tile_pool(name="state", bufs=1) as state,
        ):
            agin = dram.tile([SHARD, ELEM], fp16, tag="agin")
            # table halves, double-buffered by round parity
            tb = [[dram.tile([HALFROWS, ELEM], fp16, tag=f"tb{p}{h}",
                             name=f"tb{p}{h}") for h in range(2)]
                  for p in range(2)]
            strip = dram.tile([SHARD, KC], fp16, tag="strip")

            idx_t = [res.tile([128, NCH * 8], i16, tag=f"idx{h}",
                              name=f"idx_t{h}")
                     for h in range(2)]
            sel_t = [res.tile([128, NCH * WIN], fp16, tag=f"sel{h}",
                              name=f"sel_t{h}")
                     for h in range(2)]
            we_t = res.tile([128, 2, H], fp16, tag="we")
            nc.sync.dma_start(
                we_t[:], we_d[:].rearrange("(k p) n -> p k n", p=128))
            for h in range(2):
                nc.sync.dma_start(idx_t[h][:], idx_d[h][:])
                nc.sync.dma_start(sel_t[h][:], sel_d[h][:])

            r0_t = state.tile([BLK, NBLK, H], fp16, tag="r0")
            s_t = state.tile([BLK, NBLK, H], fp16, tag="s")
            r1_t = state.tile([BLK, NBLK, 2 * H], fp16, tag="r1")
            s2_t = state.tile([BLK, NBLK, 2 * H], fp16, tag="s2")

            # zero the pad region of the strip (cols 448:512)
            with tc.tile_pool(name="zpool", bufs=1) as zp:
                z_t = zp.tile([BLK, NBLK, KC - 7 * H], fp16, tag="z")
                nc.vector.memset(z_t[:], 0.0)
                nc.sync.dma_start(
                    strip[:, 7 * H:].rearrange("(b p) w -> p b w", p=BLK),
                    z_t[:])

            def allgather(par, which):
                nc.gpsimd.collective_compute(
                    "AllGather", mybir.AluOpType.bypass,
                    replica_groups=rg,
                    ins=[agin[which * HS:(which + 1) * HS, :].opt()],
                    outs=[tb[par][which].opt()],
                )

            # ---- embed: r0 = relu(x @ we) ----
            with (
                tc.tile_pool(name="xTp", bufs=1) as xp,
                tc.tile_pool(name="embps", bufs=4, space="PSUM") as pp,
            ):
                xT_t = xp.tile([128, 2, SHARD], fp16, tag="xT")
                nc.sync.dma_start(
                    xT_t[:], xT_d[:].rearrange("(k p) n -> p k n", p=128))
                for b in range(NBLK):
                    ps = pp.tile([BLK, H], fp32, tag="embps")
                    for k in range(2):
                        nc.tensor.matmul(
                            out=ps[:],
                            lhsT=xT_t[:, k, b * BLK:(b + 1) * BLK],
                            rhs=we_t[:, k, :],
                            start=(k == 0), stop=(k == 1),
                        )
                    nc.scalar.activation(r0_t[:, b, :], ps[:], ACT.Relu)
                nc.sync.dma_start(
                    agin[:, 0:H].rearrange("(b p) w -> p b w", p=BLK),
                    r0_t[:])
                nc.sync.dma_start(
                    strip[:, 0:H].rearrange("(b p) w -> p b w", p=BLK),
                    r0_t[:])

            allgather(0, 0)
            allgather(0, 1)

            def spmm(phase, W, consume, flush=None, next_par=None):
                par = (phase - 1) % 2
                t0, t1 = tb[par]
                with (
                    tc.tile_pool(name=f"g0{phase}", bufs=KDELAY + 5) as gp0,
                    tc.tile_pool(name=f"g1{phase}", bufs=8) as gp1,
                    tc.tile_pool(name=f"ps{phase}", bufs=6, space="PSUM") as pp,
                ):
                    g0s = {}
                    for c in range(NCALL + KDELAY):
                        if c < NCALL:
                            g0s[c] = gp0.tile(
                                [128, SB * NWPB * REP, ELEM], fp16,
                                tag="g0", name="g_t0")
                            nc.gpsimd.dma_gather(
                                g0s[c][:], t0[:],
                                idx_t[0][:, c * (CALL_IDX // 16):
                                         (c + 1) * (CALL_IDX // 16)],
                                num_idxs=CALL_IDX, num_idxs_reg=CALL_IDX,
                                elem_size=ELEM, single_packet=True,
                                queue_num=c % 4,
                            )
                        cc = c - KDELAY
                        if cc < 0:
                            continue
                        g1 = gp1.tile(
                            [128, SB * NWPB * REP, ELEM], fp16,
                            tag="g1", name="g_t1")
                        nc.gpsimd.dma_gather(
                            g1[:], t1[:],
                            idx_t[1][:, cc * (CALL_IDX // 16):
                                     (cc + 1) * (CALL_IDX // 16)],
                            num_idxs=CALL_IDX, num_idxs_reg=CALL_IDX,
                            elem_size=ELEM, single_packet=True,
                            queue_num=(cc + 2) % 4,
                        )
                        g0 = g0s.pop(cc)
                        for bi in range(SB):
                            b = cc * SB + bi
                            ps = pp.tile([BLK, 2 * H], fp32, tag="ps")
                            for w in range(NWPB):
                                q0 = (b * NWPB + w) * REP
                                for j, (hh, r) in enumerate(
                                        ((0, 0), (0, 1), (1, 0), (1, 1))):
                                    q = q0 + r
                                    cq = (bi * NWPB + w) * REP + r
                                    g = g0 if hh == 0 else g1
                                    nc.tensor.matmul(
                                        out=ps[w * WIN:(w + 1) * WIN, 0:W],
                                        lhsT=sel_t[hh][:, q * WIN:
                                                       (q + 1) * WIN],
                                        rhs=g[:, cq, 0:W],
                                        start=(j == 0), stop=(j == 3),
                                    )
                            consume(b, ps)
                        if flush is not None and (cc + 1) % FLUSH_BLKS == 0:
                            flush(cc + 1 - FLUSH_BLKS, cc + 1)
                        if next_par is not None:
                            # first half of agin complete -> AG piece 0;
                            # all of agin complete -> AG piece 1
                            if cc == (NCALL // 2) - 1:
                                allgather(next_par, 0)
                            elif cc == NCALL - 1:
                                allgather(next_par, 1)

            # ---- spmm1: s = A @ r0 ----
            def consume_s(b, ps):
                nc.scalar.activation(s_t[:, b, :], ps[:, 0:H], ACT.Copy)

            def flush_s(b0, b1):
                nc.sync.dma_start(
                    agin[b0 * BLK:b1 * BLK, 0:H]
                    .rearrange("(b p) w -> p b w", p=BLK),
                    s_t[:, b0:b1, :])

            # ---- spmm2: t = A @ s ; hop1 combine ----
            def make_consume_t(cp):
                def consume_t(b, ps):
                    t16 = cp.tile([BLK, H], fp16, tag="t16")
                    nc.scalar.activation(t16[:], ps[:, 0:H], ACT.Copy)
                    r1a = cp.tile([BLK, H], fp16, tag="r1a")
                    nc.vector.tensor_tensor(
                        out=r1a[:], in0=s_t[:, b, :], in1=r0_t[:, b, :],
                        op=ALU.subtract)
                    nc.scalar.activation(r1_t[:, b, 0:H], r1a[:], ACT.Relu)
                    r1b = cp.tile([BLK, H], fp16, tag="r1b")
                    nc.vector.tensor_tensor(
                        out=r1b[:], in0=t16[:], in1=s_t[:, b, :],
                        op=ALU.subtract)
                    nc.vector.tensor_tensor(
                        out=r1b[:], in0=r1b[:], in1=r0_t[:, b, :],
                        op=ALU.subtract)
                    nc.scalar.activation(r1_t[:, b, H:2 * H], r1b[:], ACT.Relu)
                return consume_t

            def flush_t(b0, b1):
                nc.sync.dma_start(
                    agin[b0 * BLK:b1 * BLK, :]
                    .rearrange("(b p) w -> p b w", p=BLK),
                    r1_t[:, b0:b1, :])
                nc.sync.dma_start(
                    strip[b0 * BLK:b1 * BLK, H:3 * H]
                    .rearrange("(b p) w -> p b w", p=BLK),
                    r1_t[:, b0:b1, :])

            # ---- spmm3: s2 = A @ r1 ----
            def consume_s2(b, ps):
                nc.scalar.activation(s2_t[:, b, :], ps[:], ACT.Copy)

            def flush_s2(b0, b1):
                nc.sync.dma_start(
                    agin[b0 * BLK:b1 * BLK, :]
                    .rearrange("(b p) w -> p b w", p=BLK),
                    s2_t[:, b0:b1, :])

            # ---- spmm4: t2 = A @ s2 ; hop2 combine ----
            def make_consume_t2(cp):
                def consume_t2(b, ps):
                    t16 = cp.tile([BLK, 2 * H], fp16, tag="t216")
                    nc.scalar.activation(t16[:], ps[:], ACT.Copy)
                    r2a = cp.tile([BLK, 2 * H], fp16, tag="r2a")
                    nc.vector.tensor_tensor(
                        out=r2a[:], in0=s2_t[:, b, :], in1=r1_t[:, b, :],
                        op=ALU.subtract)
                    nc.scalar.activation(r2a[:], r2a[:], ACT.Relu)
                    nc.sync.dma_start(
                        strip[b * BLK:(b + 1) * BLK, 3 * H:5 * H], r2a[:])
                    r2b = cp.tile([BLK, 2 * H], fp16, tag="r2b")
                    nc.vector.tensor_tensor(
                        out=r2b[:], in0=t16[:], in1=s2_t[:, b, :],
                        op=ALU.subtract)
                    nc.vector.tensor_tensor(
                        out=r2b[:], in0=r2b[:], in1=r1_t[:, b, :],
                        op=ALU.subtract)
                    nc.scalar.activation(r2b[:], r2b[:], ACT.Relu)
                    nc.sync.dma_start(
                        strip[b * BLK:(b + 1) * BLK, 5 * H:7 * H], r2b[:])
                return consume_t2

            spmm(1, H, consume_s, flush_s, next_par=1)
            with tc.tile_pool(name="cmb1", bufs=3) as cp1:
                spmm(2, H, make_consume_t(cp1), flush_t, next_par=0)
            spmm(3, 2 * H, consume_s2, flush_s2, next_par=1)
            with tc.tile_pool(name="cmb2", bufs=3) as cp2:
                spmm(4, 2 * H, make_consume_t2(cp2))

            # ---- classify + softmax ----
            with (
                tc.tile_pool(name="wcp", bufs=1) as wcp,
                tc.tile_pool(name="rfT", bufs=1) as rp,
                tc.tile_pool(name="clsps", bufs=4, space="PSUM") as pp,
                tc.tile_pool(name="sm", bufs=4) as smp,
            ):
                wc_t = wcp.tile([128, 4, CLASS], fp16, tag="wc")
                nc.sync.dma_start(
                    wc_t[:], wc_d[:].rearrange("(k p) n -> p k n", p=128))
                rfT_t = rp.tile([128, 4, SHARD], fp16, tag="rfT")
                for k in range(4):
                    nc.sync.dma_start(
                        out=rfT_t[:, k, :],
                        in_=strip[:, k * 128:(k + 1) * 128],
                        transpose=True,
                    )
                for b in range(NBLK):
                    ps = pp.tile([BLK, CLASS], fp32, tag="clsps")
                    for k in range(4):
                        nc.tensor.matmul(
                            out=ps[:],
                            lhsT=rfT_t[:, k, b * BLK:(b + 1) * BLK],
                            rhs=wc_t[:, k, :],
                            start=(k == 0), stop=(k == 3),
                        )
                    logit = smp.tile([BLK, CLASS], fp32, tag="logit")
                    mx = smp.tile([BLK, 1], fp32, tag="mx")
                    nc.vector.reduce_max(out=mx[:], in_=ps[:], axis=AX)
                    nc.vector.tensor_scalar_sub(logit[:], ps[:], mx[:])
                    nc.scalar.activation(logit[:], logit[:], ACT.Exp)
                    sm = smp.tile([BLK, 1], fp32, tag="smt")
                    nc.vector.reduce_sum(out=sm[:], in_=logit[:], axis=AX)
                    nc.vector.reciprocal(sm[:], sm[:])
                    nc.vector.tensor_scalar_mul(logit[:], logit[:], sm[:])
                    nc.sync.dma_start(
                        out_d[b * BLK:(b + 1) * BLK, :], logit[:])
    nc.compile()
    return nc


def _get_nc():
    if "nc" not in _CACHE:
        _CACHE["nc"] = _build_nc()
    return _CACHE["nc"]


def make_in_maps(x, edge_rows, edge_cols, edge_vals, w_embed, w_classify):
    core_of, slot_of, idx_arrs, sel_arrs = _preprocess(
        edge_rows, edge_cols, edge_vals)
    we16 = np.asarray(w_embed).astype(np.float16)
    wc16 = np.zeros((KC, CLASS), np.float16)
    wc16[:7 * H] = np.asarray(w_classify).astype(np.float16)
    x = np.asarray(x)
    in_maps = []
    for c in range(CORES):
        nodes = np.where(core_of == c)[0]
        xT = np.zeros((FEAT, SHARD), np.float16)
        xT[:, slot_of[nodes]] = x[nodes].astype(np.float16).T
        in_maps.append({
            "xT": xT, "we": we16, "wc": wc16,
            "idx0": np.ascontiguousarray(idx_arrs[c, 0]),
            "idx1": np.ascontiguousarray(idx_arrs[c, 1]),
            "sel0": np.ascontiguousarray(sel_arrs[c, 0]),
            "sel1": np.ascontiguousarray(sel_arrs[c, 1]),
        })
    return in_maps, core_of, slot_of


def kernel(x, edge_rows, edge_cols, edge_vals, w_embed, w_classify):
    from concourse.bass_utils import run_bass_kernel_spmd

    edge_rows = np.asarray(edge_rows).astype(np.int64)
    edge_cols = np.asarray(edge_cols).astype(np.int64)
    edge_vals = np.asarray(edge_vals).astype(np.float32)

    in_maps, core_of, slot_of = make_in_maps(
        x, edge_rows, edge_cols, edge_vals, w_embed, w_classify)
    nc = _get_nc()
    res = run_bass_kernel_spmd(nc, in_maps, list(range(CORES)))

    out = np.zeros((N, CLASS), np.float32)
    for c in range(CORES):
        nodes = np.where(core_of == c)[0]
        out[nodes] = res.results[c]["out"][slot_of[nodes]]
    return out


# revision 24
# speedup vs baseline: 1.0817x; 1.0817x over previous
"""Distributed Node-GCN forward on 8 Trainium2 NeuronCores (Bass/Tile).

Strategy (1-D node partition, v3):
  - Host reassigns the 50000 nodes to 8 cores x 6912 table slots (72 blocks
    of 96 rows).  Node-id halves map to slot halves (slots <3456 vs >=3456)
    so the per-round table replication splits into TWO AllGathers of
    [8*3456, 128] each, double-buffered by round parity and issued
    mid-previous-round so they hide behind the gather stream.
  - Each spmm = bulk dma_gather of r[col] rows (256B fp16 table rows) +
    one PE matmul per 128-edge chunk with a host-precomputed selector
    matrix SelT[128,32] performing val-scaling + segment-sum into a
    32-row window of a 96-row PSUM block.
  - Gather descriptor generation is spread over 4 SWDGE queues (4 Q7 core
    pairs run concurrently).  Col-half-1 gathers are issued KDELAY calls
    behind col-half-0 so the round's second AllGather can complete without
    stalling the gpsimd sequencer.
  - Edge structure (indices + selectors) is shared by all 4 spmms and kept
    resident in SBUF.
"""
import numpy as np

N = 50000
FEAT = 256
H = 64
CLASS = 64
CORES = 8
BLK = 96            # PSUM block rows
NBLK = 72           # blocks per core
SHARD = BLK * NBLK  # 6912 table slots per core
WIN = 32            # selector window rows
NWPB = 3            # windows per block
NWIN = NBLK * NWPB  # 216 windows per core
HWIN = NWIN // 2    # 108 windows per slot-half
HS = SHARD // 2     # 3456 slots per half
REP = 2             # chunks per (window, col-half)
NCH = NWIN * REP    # 432 chunks per (core, col-half)
HALFROWS = CORES * HS    # 27648 rows per table half
SB = 1                   # blocks per gather call
NCALL = NBLK // SB       # 72 calls per (spmm, half)
CALL_IDX = SB * NWPB * REP * 128   # 768 idxs per call
ELEM = 128               # table row elements (fp16) = 256B
KC = 512                 # padded classify contraction (448 -> 512)
KDELAY = 8               # col-half-1 gather issue delay (calls)
FLUSH_BLKS = 6           # blocks per batched agin/strip flush

_CACHE = {}


# --------------------------------------------------------------------------
# Host preprocessing
# --------------------------------------------------------------------------
def _assign_rows(edge_rows, edge_cols):
    """Assign each node to (core, slot). Nodes < N/2 get slots < HS (half 0),
    the rest slots >= HS, on any core.  Within each half, greedily balance
    per-(core,window) edge loads (split by col half) under caps."""
    col_half = (edge_cols >= (N // 2)).astype(np.int64)
    a_cnt = np.bincount(edge_rows[col_half == 0], minlength=N)
    b_cnt = np.bincount(edge_rows[col_half == 1], minlength=N)

    core_of = np.empty(N, np.int64)
    slot_of = np.empty(N, np.int64)
    cap = REP * 128

    for half, lo in ((0, 0), (1, N // 2)):
        nodes = np.arange(lo, lo + N // 2)
        order = np.argsort(-(a_cnt[nodes] + b_cnt[nodes]), kind="stable")
        nodes = nodes[order]
        nbin = CORES * HWIN
        loadA = np.zeros(nbin, np.int64)
        loadB = np.zeros(nbin, np.int64)
        rows = np.zeros(nbin, np.int64)
        for nd in nodes:
            a, b = a_cnt[nd], b_cnt[nd]
            feas = (loadA + a <= cap) & (loadB + b <= cap) & (rows < WIN)
            score = np.where(feas, np.maximum(loadA + a, loadB + b), 1 << 40)
            j = int(np.argmin(score))
            if not feas[j]:
                feas2 = rows < WIN
                score2 = np.where(
                    feas2, np.maximum(loadA + a, loadB + b), 1 << 40)
                j = int(np.argmin(score2))
                if not feas2[j]:
                    raise RuntimeError("row-capacity infeasible")
            c = j // HWIN
            w = half * HWIN + (j % HWIN)
            core_of[nd] = c
            slot_of[nd] = w * WIN + rows[j]
            loadA[j] += a
            loadB[j] += b
            rows[j] += 1
    return core_of, slot_of


def _preprocess(edge_rows, edge_cols, edge_vals):
    core_of, slot_of = _assign_rows(edge_rows, edge_cols)
    e_core = core_of[edge_rows]
    e_half = (slot_of[edge_cols] >= HS).astype(np.int64)
    e_idx = core_of[edge_cols] * HS + (slot_of[edge_cols] % HS)
    e_slot = slot_of[edge_rows]
    e_win = e_slot // WIN
    e_lr = e_slot % WIN

    idx_arrs = np.zeros((CORES, 2, 128, NCH * 8), np.int16)
    sel_arrs = np.zeros((CORES, 2, 128, NCH * WIN), np.float16)

    for c in range(CORES):
        for h in range(2):
            m = (e_core == c) & (e_half == h)
            win = e_win[m]
            order = np.argsort(win, kind="stable")
            win = win[order]
            idxv = e_idx[m][order]
            lr = e_lr[m][order]
            val = edge_vals[m][order]
            counts = np.bincount(win, minlength=NWIN)
            if counts.max() > REP * 128:
                raise RuntimeError(f"window overflow: {counts.max()}")
            starts = np.concatenate([[0], np.cumsum(counts)[:-1]])
            pos = np.arange(len(win)) - starts[win]
            q = win * REP + pos // 128
            s_in_chunk = pos % 128
            gslot = q * 128 + s_in_chunk
            idx_flat = np.zeros(NCH * 128, np.int16)
            idx_flat[gslot] = idxv.astype(np.int16)
            wrapped = idx_flat.reshape(-1, 16).T
            idx_arrs[c, h] = np.tile(wrapped, (8, 1))
            sel = np.zeros((128, NCH * WIN), np.float16)
            sel[s_in_chunk, q * WIN + lr] = val.astype(np.float16)
            sel_arrs[c, h] = sel
    return core_of, slot_of, idx_arrs, sel_arrs


# --------------------------------------------------------------------------
# Workaround: this walrus build rejects >1 sync wait on a CTRL instruction;
# split the Tile kernel-tail drain's waits across multiple drains.
# --------------------------------------------------------------------------
def _patch_tile_drain():
    import concourse.tile as tile
    import concourse.mybir as mybir
    from concourse.vector_clock import ScopedClock

    if getattr(tile.TileContext, "_drain_split_patched", False):
        return

    def _patched(self, tick_clock, wait_clock):
        nc = self.nc
        drain_inst = nc.sync.drain()
        wait_clock.add_sem_waits(
            drain_inst.ins, ScopedClock({None: tick_clock.global_clock})
        )
        si = drain_inst.ins.sync_info
        if si is not None and len(si.on_wait) > 1:
            waits = list(si.on_wait)
            drain_inst.ins.sync_info = mybir.SyncInfo(
                on_wait=waits[:1], on_update=list(si.on_update))
            for i in range(1, len(waits)):
                extra = nc.sync.drain()
                extra.ins.sync_info = mybir.SyncInfo(
                    on_wait=[waits[i]], on_update=[])
        nc.all_engine_barrier()
        assert self.sems is not None
        popped = nc._tile_sem_poison_stack.pop()
        assert popped is self._sem_poison
        nc.clear_and_free_semaphores(list(self.sems.allocated().values()))
        nc.all_engine_barrier()

    tile.TileContext._drain_and_barrier = _patched
    tile.TileContext._drain_split_patched = True


# --------------------------------------------------------------------------
# Device kernel
# --------------------------------------------------------------------------
def _build_nc():
    import concourse.bacc as bacc
    import concourse.mybir as mybir
    import concourse.tile as tile
    from concourse import library_config

    _patch_tile_drain()

    fp16 = mybir.dt.float16
    fp32 = mybir.dt.float32
    i16 = mybir.dt.int16
    AX = mybir.AxisListType.X
    ALU = mybir.AluOpType
    ACT = mybir.ActivationFunctionType

    nc = bacc.Bacc("TRN2", target_bir_lowering=False, debug=False,
                   num_devices=CORES, num_swdge_queues=4)
    xT_d = nc.dram_tensor("xT", [FEAT, SHARD], fp16, kind="ExternalInput")
    we_d = nc.dram_tensor("we", [FEAT, H], fp16, kind="ExternalInput")
    wc_d = nc.dram_tensor("wc", [KC, CLASS], fp16, kind="ExternalInput")
    idx_d = [nc.dram_tensor(f"idx{h}", [128, NCH * 8], i16,
                            kind="ExternalInput") for h in range(2)]
    sel_d = [nc.dram_tensor(f"sel{h}", [128, NCH * WIN], fp16,
                            kind="ExternalInput") for h in range(2)]
    out_d = nc.dram_tensor("out", [SHARD, CLASS], fp32, kind="ExternalOutput")

    rg = [list(range(CORES))]

    with tile.TileContext(nc) as tc:
        nc.gpsimd.load_library(library_config.mlp)
        with (
            tc.tile_pool(name="dram", bufs=1, space="DRAM") as dram,
            tc.tile_pool(name="resident", bufs=1) as res,
            tc.tile_pool(name="state", bufs=1) as state,
        ):
            agin = dram.tile([SHARD, ELEM], fp16, tag="agin")
            # table halves, double-buffered by round parity
            tb = [[dram.tile([HALFROWS, ELEM], fp16, tag=f"tb{p}{h}",
                             name=f"tb{p}{h}") for h in range(2)]
                  for p in range(2)]
            strip = dram.tile([SHARD, KC], fp16, tag="strip")

            idx_t = [res.tile([128, NCH * 8], i16, tag=f"idx{h}",
                              name=f"idx_t{h}")
                     for h in range(2)]
            sel_t = [res.tile([128, NCH * WIN], fp16, tag=f"sel{h}",
                              name=f"sel_t{h}")
                     for h in range(2)]
            we_t = res.tile([128, 2, H], fp16, tag="we")
            nc.sync.dma_start(
                we_t[:], we_d[:].rearrange("(k p) n -> p k n", p=128))
            for h in range(2):
                nc.sync.dma_start(idx_t[h][:], idx_d[h][:])
                nc.sync.dma_start(sel_t[h][:], sel_d[h][:])

            r0_t = state.tile([BLK, NBLK, H], fp16, tag="r0")
            s_t = state.tile([BLK, NBLK, H], fp16, tag="s")
            r1_t = state.tile([BLK, NBLK, 2 * H], fp16, tag="r1")
            s2_t = state.tile([BLK, NBLK, 2 * H], fp16, tag="s2")

            # zero the pad region of the strip (cols 448:512)
            with tc.tile_pool(name="zpool", bufs=1) as zp:
                z_t = zp.tile([BLK, NBLK, KC - 7 * H], fp16, tag="z")
                nc.vector.memset(z_t[:], 0.0)
                nc.sync.dma_start(
                    strip[:, 7 * H:].rearrange("(b p) w -> p b w", p=BLK),
                    z_t[:])

            def allgather(par, which):
                nc.gpsimd.collective_compute(
                    "AllGather", mybir.AluOpType.bypass,
                    replica_groups=rg,
                    ins=[agin[which * HS:(which + 1) * HS, :].opt()],
                    outs=[tb[par][which].opt()],
                )

            # ---- embed: r0 = relu(x @ we) ----
            with (
                tc.tile_pool(name="xTp", bufs=1) as xp,
                tc.tile_pool(name="embps", bufs=4, space="PSUM") as pp,
            ):
                xT_t = xp.tile([128, 2, SHARD], fp16, tag="xT")
                nc.sync.dma_start(
                    xT_t[:], xT_d[:].rearrange("(k p) n -> p k n", p=128))
                for b in range(NBLK):
                    ps = pp.tile([BLK, H], fp32, tag="embps")
                    for k in range(2):
                        nc.tensor.matmul(
                            out=ps[:],
                            lhsT=xT_t[:, k, b * BLK:(b + 1) * BLK],
                            rhs=we_t[:, k, :],
                            start=(k == 0), stop=(k == 1),
                        )
                    nc.scalar.activation(r0_t[:, b, :], ps[:], ACT.Relu)
                nc.sync.dma_start(
                    agin[:, 0:H].rearrange("(b p) w -> p b w", p=BLK),
                    r0_t[:])
                nc.sync.dma_start(
                    strip[:, 0:H].rearrange("(b p) w -> p b w", p=BLK),
                    r0_t[:])

            allgather(0, 0)
            allgather(0, 1)

            def spmm(phase, W, consume, flush=None, next_par=None):
                par = (phase - 1) % 2
                t0, t1 = tb[par]
                with (
                    tc.tile_pool(name=f"g0{phase}", bufs=KDELAY + 5) as gp0,
                    tc.tile_pool(name=f"g1{phase}", bufs=6) as gp1,
                    tc.tile_pool(name=f"ps{phase}", bufs=6, space="PSUM") as pp,
                ):
                    g0s = {}
                    for c in range(NCALL + KDELAY):
                        if c < NCALL:
                            g0s[c] = gp0.tile(
                                [128, SB * NWPB * REP, ELEM], fp16,
                                tag="g0", name="g_t0")
                            nc.gpsimd.dma_gather(
                                g0s[c][:], t0[:],
                                idx_t[0][:, c * (CALL_IDX // 16):
                                         (c + 1) * (CALL_IDX // 16)],
                                num_idxs=CALL_IDX, num_idxs_reg=CALL_IDX,
                                elem_size=ELEM, single_packet=True,
                                queue_num=c % 4,
                            )
                        cc = c - KDELAY
                        if cc < 0:
                            continue
                        g1 = gp1.tile(
                            [128, SB * NWPB * REP, ELEM], fp16,
                            tag="g1", name="g_t1")
                        nc.gpsimd.dma_gather(
                            g1[:], t1[:],
                            idx_t[1][:, cc * (CALL_IDX // 16):
                                     (cc + 1) * (CALL_IDX // 16)],
                            num_idxs=CALL_IDX, num_idxs_reg=CALL_IDX,
                            elem_size=ELEM, single_packet=True,
                            queue_num=(cc + 2) % 4,
                        )
                        g0 = g0s.pop(cc)
                        for bi in range(SB):
                            b = cc * SB + bi
                            ps = pp.tile([BLK, 2 * H], fp32, tag="ps")
                            for w in range(NWPB):
                                q0 = (b * NWPB + w) * REP
                                for j, (hh, r) in enumerate(
                                        ((0, 0), (0, 1), (1, 0), (1, 1))):
                                    q = q0 + r
                                    cq = (bi * NWPB + w) * REP + r
                                    g = g0 if hh == 0 else g1
                                    nc.tensor.matmul(
                                        out=ps[w * WIN:(w + 1) * WIN, 0:W],
                                        lhsT=sel_t[hh][:, q * WIN:
                                                       (q + 1) * WIN],
                                        rhs=g[:, cq, 0:W],
                                        start=(j == 0), stop=(j == 3),
                                    )
                            consume(b, ps)
                        if flush is not None and (cc + 1) % FLUSH_BLKS == 0:
                            flush(cc + 1 - FLUSH_BLKS, cc + 1)
                        if next_par is not None:
                            # first half of agin complete -> AG piece 0;
                            # all of agin complete -> AG piece 1
                            if cc == (NCALL // 2) - 1:
                                allgather(next_par, 0)
                            elif cc == NCALL - 1:
                                allgather(next_par, 1)

            # ---- spmm1: s = A @ r0 ----
            def consume_s(b, ps):
                nc.scalar.activation(s_t[:, b, :], ps[:, 0:H], ACT.Copy)

            def flush_s(b0, b1):
                nc.sync.dma_start(
                    agin[b0 * BLK:b1 * BLK, 0:H]
                    .rearrange("(b p) w -> p b w", p=BLK),
                    s_t[:, b0:b1, :])

            # ---- spmm2: t = A @ s ; hop1 combine ----
            def make_consume_t(cp):
                def consume_t(b, ps):
                    t16 = cp.tile([BLK, H], fp16, tag="t16")
                    nc.scalar.activation(t16[:], ps[:, 0:H], ACT.Copy)
                    r1a = cp.tile([BLK, H], fp16, tag="r1a")
                    nc.vector.tensor_tensor(
                        out=r1a[:], in0=s_t[:, b, :], in1=r0_t[:, b, :],
                        op=ALU.subtract)
                    nc.scalar.activation(r1_t[:, b, 0:H], r1a[:], ACT.Relu)
                    r1b = cp.tile([BLK, H], fp16, tag="r1b")
                    nc.vector.tensor_tensor(
                        out=r1b[:], in0=t16[:], in1=s_t[:, b, :],
                        op=ALU.subtract)
                    nc.vector.tensor_tensor(
                        out=r1b[:], in0=r1b[:], in1=r0_t[:, b, :],
                        op=ALU.subtract)
                    nc.scalar.activation(r1_t[:, b, H:2 * H], r1b[:], ACT.Relu)
                return consume_t

            def flush_t(b0, b1):
                nc.sync.dma_start(
                    agin[b0 * BLK:b1 * BLK, :]
                    .rearrange("(b p) w -> p b w", p=BLK),
                    r1_t[:, b0:b1, :])
                nc.sync.dma_start(
                    strip[b0 * BLK:b1 * BLK, H:3 * H]
                    .rearrange("(b p) w -> p b w", p=BLK),
                    r1_t[:, b0:b1, :])

            # ---- spmm3: s2 = A @ r1 ----
            def consume_s2(b, ps):
                nc.scalar.activation(s2_t[:, b, :], ps[:], ACT.Copy)

            def flush_s2(b0, b1):
                nc.sync.dma_start(
                    agin[b0 * BLK:b1 * BLK, :]
                    .rearrange("(b p) w -> p b w", p=BLK),
                    s2_t[:, b0:b1, :])

            # ---- spmm4: t2 = A @ s2 ; hop2 combine ----
            def make_consume_t2(cp):
                def consume_t2(b, ps):
                    t16 = cp.tile([BLK, 2 * H], fp16, tag="t216")
                    nc.scalar.activation(t16[:], ps[:], ACT.Copy)
                    r2a = cp.tile([BLK, 2 * H], fp16, tag="r2a")
                    nc.vector.tensor_tensor(
                        out=r2a[:], in0=s2_t[:, b, :], in1=r1_t[:, b, :],
                        op=ALU.subtract)
                    nc.scalar.activation(r2a[:], r2a[:], ACT.Relu)
                    nc.sync.dma_start(
                        strip[b * BLK:(b + 1) * BLK, 3 * H:5 * H], r2a[:])
                    r2b = cp.tile([BLK, 2 * H], fp16, tag="r2b")
                    nc.vector.tensor_tensor(
                        out=r2b[:], in0=t16[:], in1=s2_t[:, b, :],
                        op=ALU.subtract)
                    nc.vector.tensor_tensor(
                        out=r2b[:], in0=r2b[:], in1=r1_t[:, b, :],
                        op=ALU.subtract)
                    nc.scalar.activation(r2b[:], r2b[:], ACT.Relu)
                    nc.sync.dma_start(
                        strip[b * BLK:(b + 1) * BLK, 5 * H:7 * H], r2b[:])
                return consume_t2

            spmm(1, H, consume_s, flush_s, next_par=1)
            with tc.tile_pool(name="cmb1", bufs=3) as cp1:
                spmm(2, H, make_consume_t(cp1), flush_t, next_par=0)
            spmm(3, 2 * H, consume_s2, flush_s2, next_par=1)
            with tc.tile_pool(name="cmb2", bufs=3) as cp2:
                spmm(4, 2 * H, make_consume_t2(cp2))

            # ---- classify + softmax ----
            with (
                tc.tile_pool(name="wcp", bufs=1) as wcp,
                tc.tile_pool(name="rfT", bufs=1) as rp,
                tc.tile_pool(name="clsps", bufs=4, space="PSUM") as pp,
                tc.tile_pool(name="sm", bufs=4) as smp,
            ):
                wc_t = wcp.tile([128, 4, CLASS], fp16, tag="wc")
                nc.sync.dma_start(
                    wc_t[:], wc_d[:].rearrange("(k p) n -> p k n", p=128))
                rfT_t = rp.tile([128, 4, SHARD], fp16, tag="rfT")
                for k in range(4):
                    nc.sync.dma_start(
                        out=rfT_t[:, k, :],
                        in_=strip[:, k * 128:(k + 1) * 128],
                        transpose=True,
                    )
                for b in range(NBLK):
                    ps = pp.tile([BLK, CLASS], fp32, tag="clsps")
                    for k in range(4):
                        nc.tensor.matmul(
                            out=ps[:],
                            lhsT=rfT_t[:, k, b * BLK:(b + 1) * BLK],
                            rhs=wc_t[:, k, :],
                            start=(k == 0), stop=(k == 3),
                        )
                    logit = smp.tile([BLK, CLASS], fp32, tag="logit")
                    mx = smp.tile([BLK, 1], fp32, tag="mx")
                    nc.vector.reduce_max(out=mx[:], in_=ps[:], axis=AX)
                    nc.vector.tensor_scalar_sub(logit[:], ps[:], mx[:])
                    nc.scalar.activation(logit[:], logit[:], ACT.Exp)
                    sm = smp.tile([BLK, 1], fp32, tag="smt")
                    nc.vector.reduce_sum(out=sm[:], in_=logit[:], axis=AX)
                    nc.vector.reciprocal(sm[:], sm[:])
                    nc.vector.tensor_scalar_mul(logit[:], logit[:], sm[:])
                    nc.sync.dma_start(
                        out_d[b * BLK:(b + 1) * BLK, :], logit[:])
    nc.compile()
    return nc


def _get_nc():
    if "nc" not in _CACHE:
        _CACHE["nc"] = _build_nc()
    return _CACHE["nc"]


def make_in_maps(x, edge_rows, edge_cols, edge_vals, w_embed, w_classify):
    core_of, slot_of, idx_arrs, sel_arrs = _preprocess(
        edge_rows, edge_cols, edge_vals)
    we16 = np.asarray(w_embed).astype(np.float16)
    wc16 = np.zeros((KC, CLASS), np.float16)
    wc16[:7 * H] = np.asarray(w_classify).astype(np.float16)
    x = np.asarray(x)
    in_maps = []
    for c in range(CORES):
        nodes = np.where(core_of == c)[0]
        xT = np.zeros((FEAT, SHARD), np.float16)
        xT[:, slot_of[nodes]] = x[nodes].astype(np.float16).T
        in_maps.append({
            "xT": xT, "we": we16, "wc": wc16,
            "idx0": np.ascontiguousarray(idx_arrs[c, 0]),
            "idx1": np.ascontiguousarray(idx_arrs[c, 1]),
            "sel0": np.ascontiguousarray(sel_arrs[c, 0]),
            "sel1": np.ascontiguousarray(sel_arrs[c, 1]),
        })
    return in_maps, core_of, slot_of


def kernel(x, edge_rows, edge_cols, edge_vals, w_embed, w_classify):
    from concourse.bass_utils import run_bass_kernel_spmd

    edge_rows = np.asarray(edge_rows).astype(np.int64)
    edge_cols = np.asarray(edge_cols).astype(np.int64)
    edge_vals = np.asarray(edge_vals).astype(np.float32)

    in_maps, core_of, slot_of = make_in_maps(
        x, edge_rows, edge_cols, edge_vals, w_embed, w_classify)
    nc = _get_nc()
    res = run_bass_kernel_spmd(nc, in_maps, list(range(CORES)))

    out = np.zeros((N, CLASS), np.float32)
    for c in range(CORES):
        nodes = np.where(core_of == c)[0]
        out[nodes] = res.results[c]["out"][slot_of[nodes]]
    return out
